# revision 1
# baseline (speedup 1.0000x reference)
"""Causal self-attention with RoPE (B=4, T=2048, 16 heads x 64 dim) on 8 TRN2 cores.

Sharding: core c = (batch b = c//2, head-group g = c%2). Each core computes the
attention output of its 8 heads for its batch plus the partial output
projection; the host sums the two head-group partials per batch.

Per-core device program (all matmul operands fp16, fp32 accumulation):
  B. qkv projection. q,k produced transposed (feature rows x tokens) with a
     host-side row permutation of Wq/Wk so that each 128-row tile holds the
     rope real/imag halves 16 rows apart inside 32-row blocks; RoPE is then
     4 DVE ops per tile. v is produced in natural (token x feature) layout
     and stored with a fused ones column per head (65-wide slots) so the
     softmax denominator falls out of the PV matmul as row 64.
  C. attention, S^T orientation: S^T[tk, tq] tiles (128 x 512) accumulate in
     PSUM from K=64 fp16 matmuls (banded: diagonal chunks skip their
     causally-dead left columns). exp on ScalarE (scale=1/8, bias=-2 folded
     in; the bias cancels in the softmax normalization) writes fp16 expS.
     The causal triangle is masked by a DVE multiply with a precomputed 0/1
     tile. PV: o^T[d(+denom), tq] accumulates over tk in PSUM. Normalize:
     batched DVE reciprocal (fp32), fp16 convert, row-scatter DMA, K=1 PE
     matmul partition-broadcast, DVE multiply producing fp16 attnT
     (feature x token) - exactly the lhsT layout the projection needs.
  E. output projection into y (token x 1024) fp16, DMA'd out; the host sums
     the two head-group partials in fp32.
"""

import math

import numpy as np

import concourse.bass as bass
import concourse.mybir as mybir
import concourse.tile as tile
from concourse import bass_utils
from concourse.vector_clock import ScopedClock

# ---------------------------------------------------------------------------
# Workaround for a walrus/bass version skew: the walrus build in this image
# rejects a Drain carrying more than one sync-wait command. TileContext's exit
# barrier attaches one wait per ticked logical proc to a single SP drain;
# spread them across one-wait-per-NOP instructions ahead of the drain.
# ---------------------------------------------------------------------------


_orig_add_instruction = tile.TileContext._add_instruction


def _split_waits_add_instruction(self, inst):
    si = getattr(inst, "sync_info", None)
    if si is not None and len(si.on_wait) > 1:
        waits = list(si.on_wait)
        for w in waits[:-1]:
            nop = mybir.InstNoOp(
                name=self.nc.get_next_instruction_name(),
                engine=inst.engine,
                sync_info=mybir.SyncInfo(on_wait=[w], on_update=[]),
                bass_nofuse=True,
            )
            _orig_add_instruction(self, nop)
        inst.sync_info = mybir.SyncInfo(on_wait=[waits[-1]],
                                        on_update=list(si.on_update))
    _orig_add_instruction(self, inst)


tile.TileContext._add_instruction = _split_waits_add_instruction


def _split_waits_drain_and_barrier(self, tick_clock, wait_clock):
    probe = self.nc.sync.nop()
    wait_clock.add_sem_waits(probe.ins, ScopedClock({None: tick_clock.global_clock}))
    si = probe.ins.sync_info
    waits = list(si.on_wait) if si is not None else []
    updates = list(si.on_update) if si is not None else []
    if len(waits) > 1:
        probe.ins.sync_info = mybir.SyncInfo(on_wait=waits[:1], on_update=updates)
        for w in waits[1:]:
            n = self.nc.sync.nop()
            n.ins.sync_info = mybir.SyncInfo(on_wait=[w], on_update=[])
    self.nc.sync.drain()

    self.nc.all_engine_barrier()
    assert self.sems is not None
    popped = self.nc._tile_sem_poison_stack.pop()
    assert popped is self._sem_poison
    self.nc.clear_and_free_semaphores(list(self.sems.allocated().values()))
    self.nc.all_engine_barrier()


tile.TileContext._drain_and_barrier = _split_waits_drain_and_barrier

# ---------------------------------------------------------------------------
# Problem constants (hardcoded per the harness contract).
# ---------------------------------------------------------------------------
B, T, C = 4, 2048, 1024
N_HEAD, HEAD_DIM = 16, 64
N_CORES = 8
HL = 8          # heads per core (head-group)
F = HL * HEAD_DIM  # 512 local q/k/v features
SCALE = 1.0 / math.sqrt(HEAD_DIM)
EXP_BIAS = -2.0  # folded into exp; cancels in the softmax normalization

F16 = mybir.dt.float16
F32 = mybir.dt.float32


def _build_program():
    nc = bass.Bass("TRN2", target_bir_lowering=False, debug=False, num_devices=1)

    xT = nc.dram_tensor("xT", [C, T], F16, kind="ExternalInput")
    wq = nc.dram_tensor("wq", [C, F], F16, kind="ExternalInput")
    wk = nc.dram_tensor("wk", [C, F], F16, kind="ExternalInput")
    wv = nc.dram_tensor("wv", [C, F], F16, kind="ExternalInput")
    wp = nc.dram_tensor("wp", [F, C], F16, kind="ExternalInput")
    cosP = nc.dram_tensor("cosP", [32, T], F16, kind="ExternalInput")
    sinP = nc.dram_tensor("sinP", [32, T], F16, kind="ExternalInput")
    md0 = nc.dram_tensor("md0", [128, 128], F16, kind="ExternalInput")
    sgn = nc.dram_tensor("sgn", [128, 1], F32, kind="ExternalInput")
    y = nc.dram_tensor("y", [T, C], F16, kind="ExternalOutput")

    Exp = mybir.ActivationFunctionType.Exp
    NKC = C // 128   # 8 contraction chunks
    NTQ = T // 512   # 4 query-column tiles

    with tile.TileContext(nc) as tc:
        with (
            tc.tile_pool(name="const", bufs=1) as const,
            tc.tile_pool(name="work", bufs=3) as wk_pool,
            tc.tile_pool(name="psum", bufs=2, space="PSUM") as psum,
        ):
            # ---------------- input loads ----------------
            # five DMA queues; x is loaded in 512-column chunks so the first
            # qkv group's accumulation chain only waits on ~2MB, and chunk c
            # lands just before tq-block c consumes it.
            dma_engines = [nc.sync, nc.scalar, nc.gpsimd]
            _dma_i = [0]

            def load(t, src):
                eng = dma_engines[_dma_i[0] % len(dma_engines)]
                _dma_i[0] += 1
                eng.dma_start(out=t, in_=src)

            xT_sb = [const.tile([128, T], F16, tag=f"xt{i}", name=f"xt{i}")
                     for i in range(NKC)]
            wq_sb, wk_sb, wv_sb = [], [], []
            for name, lst in (("wq", wq_sb), ("wk", wk_sb), ("wv", wv_sb)):
                for i in range(NKC):
                    lst.append(const.tile([128, F], F16, tag=f"{name}{i}",
                                          name=f"{name}{i}"))
            sgn_sb = const.tile([128, 1], F32, tag="sgn", name="sgn_sb")
            load(sgn_sb, sgn.ap())
            md0_sb = const.tile([128, 128], F16, tag="md0", name="md0")
            load(md0_sb, md0.ap())
            c32 = const.tile([32, T], F16, tag="c32", name="c32")
            load(c32, cosP.ap())
            s32 = const.tile([32, T], F16, tag="s32", name="s32")
            load(s32, sinP.ap())
            # expand the 32-freq tables to the 128-row rope layout
            # (blk = [f0-15, f0-15, f16-31, f16-31] twice)
            cos_sb = const.tile([128, T], F16, tag="cos", name="cos")
            sin_sb = const.tile([128, T], F16, tag="sin", name="sin")
            for g in range(8):
                half = 16 * ((g % 4) // 2)
                nc.sync.dma_start(out=cos_sb[16 * g:16 * g + 16, :],
                                  in_=c32[half:half + 16, :])
                nc.sync.dma_start(out=sin_sb[16 * g:16 * g + 16, :],
                                  in_=s32[half:half + 16, :])
            for i in range(NKC):
                load(wq_sb[i], wq.ap()[i * 128:(i + 1) * 128, :])
            for i in range(NKC):
                load(xT_sb[i][:, 0:512], xT.ap()[i * 128:(i + 1) * 128, 0:512])
            for i in range(NKC):
                load(wk_sb[i], wk.ap()[i * 128:(i + 1) * 128, :])
            for i in range(NKC):
                load(wv_sb[i], wv.ap()[i * 128:(i + 1) * 128, :])
            for i in range(NKC):
                load(xT_sb[i][:, 512:1024],
                     xT.ap()[i * 128:(i + 1) * 128, 512:1024])
            for i in range(NKC):
                load(xT_sb[i][:, 1024:1536],
                     xT.ap()[i * 128:(i + 1) * 128, 1024:1536])
            for i in range(NKC):
                load(xT_sb[i][:, 1536:2048],
                     xT.ap()[i * 128:(i + 1) * 128, 1536:2048])
            wp_sb = []
            for i in range(F // 128):
                t = const.tile([128, C], F16, tag=f"wp{i}", name=f"wp{i}")
                load(t, wp.ap()[i * 128:(i + 1) * 128, :])
                wp_sb.append(t)

            qT_sb = [const.tile([128, T], F16, tag=f"q{j}", name=f"qT{j}") for j in range(4)]
            kT_sb = [const.tile([128, T], F16, tag=f"k{j}", name=f"kT{j}") for j in range(4)]
            bias_sb = const.tile([128, 1], F32, tag="bias", name="bias_sb")
            nc.gpsimd.memset(bias_sb, EXP_BIAS)
            # v tiles: 8 head-slots of 65 (64 features + ones column)
            v_sb = []
            for i in range(T // 128):
                t = const.tile([128, HL * 65], F16, tag=f"v{i}", name=f"v{i}")
                nc.gpsimd.memset(t, 1.0)
                v_sb.append(t)
            attnT_sb = [const.tile([128, T], F16, tag=f"at{j}", name=f"at{j}") for j in range(4)]
            # ones on every partition so K=1 broadcast matmuls can read their
            # lhsT at base partitions 0/32/64/96 (striped denominators)
            ones_sb = const.tile([128, 64], F16, tag="ones", name="ones_sb")
            nc.gpsimd.memset(ones_sb, 1.0)

            shuf_mask = list(range(16, 32)) + list(range(16))
            mult = mybir.AluOpType.mult
            add = mybir.AluOpType.add

            def qk_group(w_sb, dst_sb, j, tqb):
                # one qkv matmul group + fp16 rope (ScalarE evacuates PSUM so
                # every DVE rope op runs in the 2-byte 2x mode)
                ps = psum.tile([128, 512], F32, tag="mm", name="mm_ps")
                for kc in range(NKC):
                    nc.tensor.matmul(
                        ps,
                        lhsT=w_sb[kc][:, j * 128:(j + 1) * 128],
                        rhs=xT_sb[kc][:, tqb * 512:(tqb + 1) * 512],
                        start=(kc == 0), stop=(kc == NKC - 1),
                    )
                qf = wk_pool.tile([128, 512], F16, tag="qf", name="qf", bufs=3)
                nc.scalar.copy(qf, ps)
                cs = cos_sb[:, tqb * 512:(tqb + 1) * 512]
                sn = sin_sb[:, tqb * 512:(tqb + 1) * 512]
                u = wk_pool.tile([128, 512], F16, tag="u", name="u", bufs=2)
                w_ = wk_pool.tile([128, 512], F16, tag="w", name="w_", bufs=2)
                nc.vector.tensor_mul(u, qf, cs)       # R*c / I*c
                nc.vector.tensor_mul(w_, qf, sn)      # R*s / I*s
                w2 = wk_pool.tile([128, 512], F16, tag="w2", name="w2", bufs=2)
                nc.vector.stream_shuffle(w2, w_, shuf_mask)
                dst = dst_sb[j][:, tqb * 512:(tqb + 1) * 512]
                # out = u + sgn*w2: rows R' = R*c - I*s, I' = I*c + R*s
                nc.vector.scalar_tensor_tensor(dst, w2, sgn_sb, u, mult, add)

            def v_group(tt):
                ps = psum.tile([128, 512], F32, tag="mm", name="mm_ps")
                for kc in range(NKC):
                    nc.tensor.matmul(
                        ps,
                        lhsT=xT_sb[kc][:, tt * 128:(tt + 1) * 128],
                        rhs=wv_sb[kc],
                        start=(kc == 0), stop=(kc == NKC - 1),
                    )
                dst = v_sb[tt].rearrange("p (h e) -> p h e", e=65)[:, :, 0:64]
                src = ps.rearrange("p (h e) -> p h e", e=64)
                nc.scalar.copy(dst, src)

            def attn_unit(jq, jp, dn_map, o_sbs, pop_filler):
                # heads (2jp, 2jp+1) for query window jq: QK -> exp -> mask ->
                # PV; denominators land striped in dn_sb at partition
                # 32*(l%4), column half l//4.
                n_tk = 4 * (jq + 1)
                o_ps = [psum.tile([65, 512], F32, tag="o", name="o_ps")
                        for _ in range(2)]
                for tg in range(n_tk // 2):
                    def _off(kb):
                        dband = kb - 4 * jq
                        return 128 * dband if dband > 0 else 0

                    s_ps = [psum.tile([128, 1024], F32, tag="s", name="s_ps")
                            for _ in range(2)]
                    for c in range(2):
                        kb = tg * 2 + c
                        off = _off(kb)
                        for hh in range(2):
                            sl = s_ps[hh][:, c * 512 + off:(c + 1) * 512]
                            hb = hh * 64
                            nc.tensor.matmul(
                                sl,
                                lhsT=kT_sb[jp][hb:hb + 64,
                                               kb * 128:(kb + 1) * 128],
                                rhs=qT_sb[jp][hb:hb + 64,
                                              jq * 512 + off:(jq + 1) * 512],
                                start=True, stop=True,
                            )
                    exp_t = []
                    offs = [_off(tg * 2), _off(tg * 2 + 1)]
                    for hh in range(2):
                        e = wk_pool.tile([128, 1024], F16, tag="expS",
                                         name="expS", bufs=3)
                        if offs[0] == 0 and offs[1] == 0:
                            nc.scalar.activation(e, s_ps[hh], Exp,
                                                 bias=bias_sb[:, 0:1],
                                                 scale=SCALE)
                        else:
                            for c in range(2):
                                sl = slice(c * 512 + offs[c], (c + 1) * 512)
                                nc.scalar.activation(e[:, sl], s_ps[hh][:, sl],
                                                     Exp,
                                                     bias=bias_sb[:, 0:1],
                                                     scale=SCALE)
                        exp_t.append(e)
                    for c in range(2):
                        kb = tg * 2 + c
                        dband = kb - 4 * jq
                        if 0 <= dband <= 3:
                            off = 128 * dband
                            for hh in range(2):
                                sl = exp_t[hh][:, c * 512 + off:
                                               c * 512 + off + 128]
                                nc.vector.tensor_mul(sl, sl, md0_sb)
                    for c in range(2):
                        kb = tg * 2 + c
                        off = _off(kb)
                        for hh in range(2):
                            l = jp * 2 + hh
                            nc.tensor.matmul(
                                o_ps[hh][:, off:512],
                                lhsT=v_sb[kb][:, l * 65:(l + 1) * 65],
                                rhs=exp_t[hh][:, c * 512 + off:(c + 1) * 512],
                                start=(kb == 0), stop=(kb == n_tk - 1),
                            )
                    pop_filler()
                for hh in range(2):
                    l = jp * 2 + hh
                    o_sb = wk_pool.tile([65, 512], F32, tag="osb",
                                        name="o_sb", bufs=10)
                    nc.vector.tensor_copy(o_sb, o_ps[hh])
                    o_sbs[l] = o_sb
                    dnt, dr = dn_map(l)
                    nc.sync.dma_start(out=dnt[dr:dr + 1, :],
                                      in_=o_sb[64:65, :])

            def norm_closures(jq, dn_sb, o_sbs, rds, heads, row0=0):
                # reciprocal over dn rows `heads`, scatter, then per head a
                # K=1 partition-broadcast matmul + normalizing DVE multiply
                out = []

                def recip_head():
                    nh = len(heads)
                    rd = wk_pool.tile([8, 512], F32, tag="rd", name="rd",
                                      bufs=2)
                    nc.vector.reciprocal(rd[0:nh, :], dn_sb[row0:row0 + nh, :])
                    rd16 = wk_pool.tile([8, 512], F16, tag="rd16",
                                        name="rd16", bufs=2)
                    nc.vector.tensor_copy(rd16[0:nh, :], rd[0:nh, :])
                    for r, l in enumerate(heads):
                        nc.sync.dma_start(
                            out=rds[32 * (l % 2):32 * (l % 2) + 1,
                                    (l // 2) * 512:(l // 2) * 512 + 512],
                            in_=rd16[r:r + 1, :])
                out.append(recip_head)

                def bc_mult(l):
                    prow = 32 * (l % 2)
                    pcol = (l // 2) * 512
                    bc_t = psum.tile([128, 512], F32, tag="mm", name="bc_ps")
                    rdb = bc_t[0:64, :]
                    nc.tensor.matmul(
                        rdb,
                        lhsT=ones_sb[prow:prow + 1, :],
                        rhs=rds[prow:prow + 1, pcol:pcol + 512],
                        start=True, stop=True,
                    )
                    at = attnT_sb[l // 2]
                    rbase = (l % 2) * 64
                    nc.vector.tensor_mul(
                        at[rbase:rbase + 64, jq * 512:(jq + 1) * 512],
                        o_sbs[l][0:64, :], rdb,
                    )
                for l in heads:
                    out.append(lambda l=l: bc_mult(l))
                return out

            def proj_closures(jq):
                out = []

                def proj_co(tt, co):
                    ps = psum.tile([128, 512], F32, tag="mm", name="mm_ps")
                    for fc in range(4):
                        nc.tensor.matmul(
                            ps,
                            lhsT=attnT_sb[fc][:, tt * 128:(tt + 1) * 128],
                            rhs=wp_sb[fc][:, co * 512:(co + 1) * 512],
                            start=(fc == 0), stop=(fc == 3),
                        )
                    ysb = wk_pool.tile([128, 512], F16, tag="ysb",
                                       name="ysb", bufs=2)
                    nc.vector.tensor_copy(ysb, ps)
                    nc.sync.dma_start(
                        out=y.ap()[tt * 128:(tt + 1) * 128,
                                   co * 512:(co + 1) * 512],
                        in_=ysb,
                    )
                for tt in range(4 * jq, 4 * jq + 4):
                    for co in range(2):
                        out.append(lambda tt=tt, co=co: proj_co(tt, co))
                return out

            # ---- interleaved schedule: qkv tq-block c feeds attention
            # wave jq=c; the next block and the previous window's
            # normalize+proj drain as per-tg fillers inside each wave so the
            # PE never idles (keeps the clock ramp hot). Block 3's j1-j3
            # q/k groups defer into wave 3 (prioritized ahead of wave-2
            # normalize) to keep the long last wave fed; wave 3's own
            # normalize splits so heads 0-3 hide inside units (3,2..3). ----
            def qk_closures(tqb, js):
                out = []
                for j in js:
                    out.append(lambda j=j: qk_group(wq_sb, qT_sb, j, tqb))
                    out.append(lambda j=j: qk_group(wk_sb, kT_sb, j, tqb))
                return out

            def v_closures(tqb):
                return [lambda tt=tt: v_group(tt)
                        for tt in range(4 * tqb, 4 * tqb + 4)]

            for f in qk_closures(0, range(4)) + v_closures(0):
                f()

            wave_fill = {
                1: [],   # norm0+proj0 prepended below
                2: [],   # norm1+proj1 prepended below
                3: [],   # block3 j1-j3 first, then norm2+proj2
            }
            pending = {}
            rds_t = {}
            for jq in range(NTQ):
                if jq == 0:
                    fillers = qk_closures(1, range(4)) + v_closures(1)
                elif jq == 1:
                    fillers = (pending[0] + qk_closures(2, range(4))
                               + v_closures(2))
                elif jq == 2:
                    fillers = (pending[1] + v_closures(3)
                               + qk_closures(3, [0]))
                else:
                    fillers = qk_closures(3, [1, 2, 3]) + pending[2]
                n_tgs = 8 * (jq + 1)
                state = {"idx": 0, "acc": 0.0, "n_tgs": n_tgs}

                def pop_filler(state=state, fillers=fillers):
                    # Bresenham spread: emit fillers evenly across the wave
                    state["acc"] += len(fillers) / state["n_tgs"]
                    while state["acc"] >= 1.0 and state["idx"] < len(fillers):
                        fillers[state["idx"]]()
                        state["idx"] += 1
                        state["acc"] -= 1.0

                dn_sb = wk_pool.tile([8, 512], F32, tag="dn", name="dn_sb",
                                     bufs=2)
                if jq == NTQ - 1:
                    dn_sb2 = wk_pool.tile([2, 512], F32, tag="dn2",
                                          name="dn_sb2", bufs=1)
                    dn_sb3 = wk_pool.tile([2, 512], F32, tag="dn3",
                                          name="dn_sb3", bufs=1)
                    dn_map = lambda l: ((dn_sb, l) if l < 4
                                        else (dn_sb2, l - 4) if l < 6
                                        else (dn_sb3, l - 6))
                else:
                    dn_map = lambda l: (dn_sb, l)
                o_sbs = {}
                rds = wk_pool.tile([33, 2048], F16, tag="rds", name="rds",
                                   bufs=2)
                for jp in range(4):
                    attn_unit(jq, jp, dn_map, o_sbs, pop_filler)
                    if jq == NTQ - 1 and jp == 1:
                        # mid-wave normalize of heads 0-3 rides the
                        # remaining units' tg stream
                        fillers.extend(
                            norm_closures(jq, dn_sb, o_sbs, rds, [0, 1, 2, 3]))
                    if jq == NTQ - 1 and jp == 2:
                        fillers.extend(
                            norm_closures(jq, dn_sb2, o_sbs, rds, [4, 5]))
                while state["idx"] < len(fillers):
                    fillers[state["idx"]]()
                    state["idx"] += 1
                if jq < NTQ - 1:
                    pending[jq] = (norm_closures(jq, dn_sb, o_sbs, rds,
                                                 list(range(8)))
                                   + proj_closures(jq))
                else:
                    for f in norm_closures(jq, dn_sb3, o_sbs, rds,
                                           [6, 7]):
                        f()
                    for f in proj_closures(jq):
                        f()
    return nc


_NC = None


def _get_nc():
    global _NC
    if _NC is None:
        _NC = _build_program()
    return _NC


def _rope_perm():
    """Row permutation applied to Wq/Wk rows (local feature order).

    Per head (64 contiguous rows, so QK^T is a single K=64 matmul):
    [R(freq 0..15), I(freq 0..15), R(freq 16..31), I(freq 16..31)] --
    R/I pairs sit 16 rows apart inside each 32-row block, which is what
    the DVE stream_shuffle (intra-32-block permute) needs for RoPE.
    """
    perm = []
    for hh in range(8):
        base = hh * HEAD_DIM
        perm.extend(base + 2 * i for i in range(16))
        perm.extend(base + 2 * i + 1 for i in range(16))
        perm.extend(base + 2 * i for i in range(16, 32))
        perm.extend(base + 2 * i + 1 for i in range(16, 32))
    return np.asarray(perm)


def _freq_rows():
    """freq index feeding each of the 128 cosP/sinP rows (2 head blocks)."""
    blk = np.array([*range(16), *range(16), *range(16, 32), *range(16, 32)])
    return np.concatenate([blk, blk])


def _core_in_map(c, x, freqs_cos, freqs_sin, w_attn, w_proj, cosP, sinP, perm):
    b, g = c // 2, c % 2
    heads = slice(g * F, (g + 1) * F)          # global q/k/v row block
    wq_rows = w_attn[0:C][heads][perm]          # (512, 1024) permuted
    wk_rows = w_attn[C:2 * C][heads][perm]
    wv_rows = w_attn[2 * C:3 * C][heads]        # natural order
    p = np.arange(128)[:, None]
    md0 = (np.arange(128)[None, :] >= p).astype(np.float16)
    return {
        "xT": np.ascontiguousarray(x[b].T).astype(np.float16),
        "wq": np.ascontiguousarray(wq_rows.T).astype(np.float16),
        "wk": np.ascontiguousarray(wk_rows.T).astype(np.float16),
        "wv": np.ascontiguousarray(wv_rows.T).astype(np.float16),
        "wp": np.ascontiguousarray(w_proj[:, heads].T).astype(np.float16),
        "cosP": cosP,
        "sinP": sinP,
        "md0": md0,
        "sgn": np.tile(np.repeat(np.float32([-1.0, 1.0]), 16), 4)[:, None],
    }


def _host_prep(freqs_cos, freqs_sin):
    cosP = np.ascontiguousarray(freqs_cos.T).astype(np.float16)
    sinP = np.ascontiguousarray(freqs_sin.T).astype(np.float16)
    return cosP, sinP, _rope_perm()


def kernel(x, freqs_cos, freqs_sin, w_attn, w_proj):
    nc = _get_nc()

    cosP, sinP, perm = _host_prep(freqs_cos, freqs_sin)
    in_maps = [
        _core_in_map(c, x, freqs_cos, freqs_sin, w_attn, w_proj,
                     cosP, sinP, perm)
        for c in range(N_CORES)
    ]

    global _last_in_maps
    _last_in_maps = in_maps
    res = bass_utils.run_bass_kernel_spmd(nc, in_maps, core_ids=list(range(N_CORES)))

    out = np.empty((B, T, C), dtype=np.float32)
    for b in range(B):
        out[b] = (res.results[2 * b]["y"].astype(np.float32)
                  + res.results[2 * b + 1]["y"].astype(np.float32))
    return out



# revision 23
# speedup vs baseline: 1.0121x; 1.0121x over previous
"""Causal self-attention with RoPE (B=4, T=2048, 16 heads x 64 dim) on 8 TRN2 cores.

Sharding: core c = (batch b = c//2, head-group g = c%2). Each core computes the
attention output of its 8 heads for its batch plus the partial output
projection; the host sums the two head-group partials per batch.

Per-core device program (all matmul operands fp16, fp32 accumulation):
  B. qkv projection. q,k produced transposed (feature rows x tokens) with a
     host-side row permutation of Wq/Wk so that each 128-row tile holds the
     rope real/imag halves 16 rows apart inside 32-row blocks; RoPE is then
     4 DVE ops per tile. v is produced in natural (token x feature) layout
     and stored with a fused ones column per head (65-wide slots) so the
     softmax denominator falls out of the PV matmul as row 64.
  C. attention, S^T orientation: S^T[tk, tq] tiles (128 x 512) accumulate in
     PSUM from K=64 fp16 matmuls (banded: diagonal chunks skip their
     causally-dead left columns). exp on ScalarE (scale=1/8, bias=-2 folded
     in; the bias cancels in the softmax normalization) writes fp16 expS.
     The causal triangle is masked by a DVE multiply with a precomputed 0/1
     tile. PV: o^T[d(+denom), tq] accumulates over tk in PSUM. Normalize:
     batched DVE reciprocal (fp32), fp16 convert, row-scatter DMA, K=1 PE
     matmul partition-broadcast, DVE multiply producing fp16 attnT
     (feature x token) - exactly the lhsT layout the projection needs.
  E. output projection into y (token x 1024) fp16, DMA'd out; the host sums
     the two head-group partials in fp32.

Startup: PE warm-up dummies cover the DMA ramp (HAM clock-gate opens before
the first real matmul); input DMA is ordered/queue-assigned by first use
(wq/wk j0 column blocks + x chunk 0 first, wp last); rope tables arrive
pre-expanded from the host.
"""

import math

import numpy as np

import concourse.bass as bass
import concourse.mybir as mybir
import concourse.tile as tile
from concourse import bass_utils
from concourse.vector_clock import ScopedClock

# ---------------------------------------------------------------------------
# Workaround for a walrus/bass version skew: the walrus build in this image
# rejects a Drain carrying more than one sync-wait command. TileContext's exit
# barrier attaches one wait per ticked logical proc to a single SP drain;
# spread them across one-wait-per-NOP instructions ahead of the drain.
# ---------------------------------------------------------------------------


_orig_add_instruction = tile.TileContext._add_instruction


def _split_waits_add_instruction(self, inst):
    si = getattr(inst, "sync_info", None)
    if si is not None and len(si.on_wait) > 1:
        waits = list(si.on_wait)
        for w in waits[:-1]:
            nop = mybir.InstNoOp(
                name=self.nc.get_next_instruction_name(),
                engine=inst.engine,
                sync_info=mybir.SyncInfo(on_wait=[w], on_update=[]),
                bass_nofuse=True,
            )
            _orig_add_instruction(self, nop)
        inst.sync_info = mybir.SyncInfo(on_wait=[waits[-1]],
                                        on_update=list(si.on_update))
    _orig_add_instruction(self, inst)


tile.TileContext._add_instruction = _split_waits_add_instruction


def _split_waits_drain_and_barrier(self, tick_clock, wait_clock):
    probe = self.nc.sync.nop()
    wait_clock.add_sem_waits(probe.ins, ScopedClock({None: tick_clock.global_clock}))
    si = probe.ins.sync_info
    waits = list(si.on_wait) if si is not None else []
    updates = list(si.on_update) if si is not None else []
    if len(waits) > 1:
        probe.ins.sync_info = mybir.SyncInfo(on_wait=waits[:1], on_update=updates)
        for w in waits[1:]:
            n = self.nc.sync.nop()
            n.ins.sync_info = mybir.SyncInfo(on_wait=[w], on_update=[])
    self.nc.sync.drain()

    self.nc.all_engine_barrier()
    assert self.sems is not None
    popped = self.nc._tile_sem_poison_stack.pop()
    assert popped is self._sem_poison
    self.nc.clear_and_free_semaphores(list(self.sems.allocated().values()))
    self.nc.all_engine_barrier()


tile.TileContext._drain_and_barrier = _split_waits_drain_and_barrier

# ---------------------------------------------------------------------------
# Problem constants (hardcoded per the harness contract).
# ---------------------------------------------------------------------------
B, T, C = 4, 2048, 1024
N_HEAD, HEAD_DIM = 16, 64
N_CORES = 8
HL = 8          # heads per core (head-group)
F = HL * HEAD_DIM  # 512 local q/k/v features
SCALE = 1.0 / math.sqrt(HEAD_DIM)
EXP_BIAS = -2.0  # folded into exp; cancels in the softmax normalization

N_DUMMY = 28    # PE warm-up matmuls covering the input-DMA ramp

F16 = mybir.dt.float16
F32 = mybir.dt.float32


def _build_program():
    nc = bass.Bass("TRN2", target_bir_lowering=False, debug=False, num_devices=1)

    xT = nc.dram_tensor("xT", [C, T], F16, kind="ExternalInput")
    wq = nc.dram_tensor("wq", [C, F], F16, kind="ExternalInput")
    wk = nc.dram_tensor("wk", [C, F], F16, kind="ExternalInput")
    wv = nc.dram_tensor("wv", [C, F], F16, kind="ExternalInput")
    wp = nc.dram_tensor("wp", [F, C], F16, kind="ExternalInput")
    cosP = nc.dram_tensor("cosP", [128, T], F16, kind="ExternalInput")
    sinP = nc.dram_tensor("sinP", [128, T], F16, kind="ExternalInput")
    md0 = nc.dram_tensor("md0", [128, 128], F16, kind="ExternalInput")
    sgn = nc.dram_tensor("sgn", [128, 1], F32, kind="ExternalInput")
    y = nc.dram_tensor("y", [T, C], F16, kind="ExternalOutput")

    Exp = mybir.ActivationFunctionType.Exp
    NKC = C // 128   # 8 contraction chunks
    NTQ = T // 512   # 4 query-column tiles

    with tile.TileContext(nc) as tc:
        with (
            tc.tile_pool(name="const", bufs=1) as const,
            tc.tile_pool(name="work", bufs=3) as wk_pool,
            tc.tile_pool(name="psum", bufs=2, space="PSUM") as psum,
        ):
            dum_sb = const.tile([128, 512], F16, tag="dum", name="dum_sb")
            nc.gpsimd.memset(dum_sb, 0.0)  # first: gates the warm-up MMs

            xT_sb = [const.tile([128, T], F16, tag=f"xt{i}", name=f"xt{i}")
                     for i in range(NKC)]
            wq_sb, wk_sb, wv_sb = [], [], []
            for name, lst in (("wq", wq_sb), ("wk", wk_sb), ("wv", wv_sb)):
                for i in range(NKC):
                    lst.append(const.tile([128, F], F16, tag=f"{name}{i}",
                                          name=f"{name}{i}"))
            sgn_sb = const.tile([128, 1], F32, tag="sgn", name="sgn_sb")
            md0_sb = const.tile([128, 128], F16, tag="md0", name="md0")
            cos_sb = const.tile([128, T], F16, tag="cos", name="cos")
            sin_sb = const.tile([128, T], F16, tag="sin", name="sin")
            wp_sb = [const.tile([128, C], F16, tag=f"wp{i}", name=f"wp{i}")
                     for i in range(F // 128)]

            qT_sb = [const.tile([128, T], F16, tag=f"q{j}", name=f"qT{j}")
                     for j in range(4)]
            kT_sb = [const.tile([128, T], F16, tag=f"k{j}", name=f"kT{j}")
                     for j in range(4)]
            bias_sb = const.tile([128, 1], F32, tag="bias", name="bias_sb")
            nc.gpsimd.memset(bias_sb, EXP_BIAS)
            # v tiles: 8 head-slots of 65 (64 features + ones column)
            v_sb = []
            for i in range(T // 128):
                t = const.tile([128, HL * 65], F16, tag=f"v{i}", name=f"v{i}")
                nc.gpsimd.memset(t, 1.0)
                v_sb.append(t)
            attnT_sb = [const.tile([128, T], F16, tag=f"at{j}", name=f"at{j}")
                        for j in range(4)]
            # ones on every partition so K=1 broadcast matmuls can read their
            # lhsT at base partitions 0/32/64/96 (striped denominators)
            ones_sb = const.tile([128, 64], F16, tag="ones", name="ones_sb")
            nc.gpsimd.memset(ones_sb, 1.0)

            # ---------------- PE warm-up dummies ----------------
            # Stream matmuls on zeros from t~0 so the HAM clock-gate opens
            # during the input-DMA ramp and the first real matmuls run at
            # 2.4 GHz. Results are never read.
            for i in range(N_DUMMY):
                ps = psum.tile([128, 512], F32, tag="mm", name="warm_ps")
                nc.tensor.matmul(ps, lhsT=dum_sb[:, 0:128], rhs=dum_sb,
                                 start=True, stop=True)

            # ---------------- input loads ----------------
            # Queue A (sync HW): latency-critical first tiles, then free for
            # mid-kernel scatters + y out. Queues B (scalar HW) / C (gpsimd
            # SW) carry the bulk, ordered by first consumption.
            QA, QB, QC = nc.sync, nc.scalar, nc.gpsimd
            QA.dma_start(out=sgn_sb, in_=sgn.ap())
            for i in range(NKC):       # wq j0 column block
                QA.dma_start(out=wq_sb[i][:, 0:128],
                             in_=wq.ap()[i * 128:(i + 1) * 128, 0:128])
            for i in range(3):         # x chunk 0
                QA.dma_start(out=xT_sb[i][:, 0:512],
                             in_=xT.ap()[i * 128:(i + 1) * 128, 0:512])
            for i in range(3, 6):
                QB.dma_start(out=xT_sb[i][:, 0:512],
                             in_=xT.ap()[i * 128:(i + 1) * 128, 0:512])
            for i in range(6, 8):
                QC.dma_start(out=xT_sb[i][:, 0:512],
                             in_=xT.ap()[i * 128:(i + 1) * 128, 0:512])
            for i in range(NKC):       # wk j0 column block
                QA.dma_start(out=wk_sb[i][:, 0:128],
                             in_=wk.ap()[i * 128:(i + 1) * 128, 0:128])
            # rope tables (first window) + mask tile early
            QB.dma_start(out=cos_sb[:, 0:512], in_=cosP.ap()[:, 0:512])
            QC.dma_start(out=sin_sb[:, 0:512], in_=sinP.ap()[:, 0:512])
            QA.dma_start(out=md0_sb, in_=md0.ap())
            for i in range(4):         # wv
                QB.dma_start(out=wv_sb[i], in_=wv.ap()[i * 128:(i + 1) * 128, :])
            for i in range(4, 8):
                QC.dma_start(out=wv_sb[i], in_=wv.ap()[i * 128:(i + 1) * 128, :])
            for i in range(4):         # wq/wk j1-3 blocks
                QB.dma_start(out=wq_sb[i][:, 128:512],
                             in_=wq.ap()[i * 128:(i + 1) * 128, 128:512])
            for i in range(4, 8):
                QC.dma_start(out=wq_sb[i][:, 128:512],
                             in_=wq.ap()[i * 128:(i + 1) * 128, 128:512])
            for i in range(4):
                QB.dma_start(out=wk_sb[i][:, 128:512],
                             in_=wk.ap()[i * 128:(i + 1) * 128, 128:512])
            for i in range(4, 8):
                QC.dma_start(out=wk_sb[i][:, 128:512],
                             in_=wk.ap()[i * 128:(i + 1) * 128, 128:512])
            QB.dma_start(out=cos_sb[:, 512:2048], in_=cosP.ap()[:, 512:2048])
            QC.dma_start(out=sin_sb[:, 512:2048], in_=sinP.ap()[:, 512:2048])
            for cchunk in range(1, 4):  # x chunks 1-3
                lo, hi = cchunk * 512, (cchunk + 1) * 512
                for i in range(4):
                    QB.dma_start(out=xT_sb[i][:, lo:hi],
                                 in_=xT.ap()[i * 128:(i + 1) * 128, lo:hi])
                for i in range(4, 8):
                    QC.dma_start(out=xT_sb[i][:, lo:hi],
                                 in_=xT.ap()[i * 128:(i + 1) * 128, lo:hi])
            for i in range(2):          # wp last (first use ~2 waves in)
                QB.dma_start(out=wp_sb[i], in_=wp.ap()[i * 128:(i + 1) * 128, :])
            for i in range(2, 4):
                QC.dma_start(out=wp_sb[i], in_=wp.ap()[i * 128:(i + 1) * 128, :])

            shuf_mask = list(range(16, 32)) + list(range(16))
            mult = mybir.AluOpType.mult
            add = mybir.AluOpType.add

            def qk_group(w_sb, dst_sb, j, tqb):
                # one qkv matmul group + fp16 rope (ScalarE evacuates PSUM so
                # every DVE rope op runs in the 2-byte 2x mode)
                ps = psum.tile([128, 512], F32, tag="mm", name="mm_ps")
                for kc in range(NKC):
                    nc.tensor.matmul(
                        ps,
                        lhsT=w_sb[kc][:, j * 128:(j + 1) * 128],
                        rhs=xT_sb[kc][:, tqb * 512:(tqb + 1) * 512],
                        start=(kc == 0), stop=(kc == NKC - 1),
                    )
                qf = wk_pool.tile([128, 512], F16, tag="qf", name="qf", bufs=3)
                nc.scalar.copy(qf, ps)
                cs = cos_sb[:, tqb * 512:(tqb + 1) * 512]
                sn = sin_sb[:, tqb * 512:(tqb + 1) * 512]
                u = wk_pool.tile([128, 512], F16, tag="u", name="u", bufs=2)
                w_ = wk_pool.tile([128, 512], F16, tag="w", name="w_", bufs=2)
                nc.vector.tensor_mul(u, qf, cs)       # R*c / I*c
                nc.vector.tensor_mul(w_, qf, sn)      # R*s / I*s
                w2 = wk_pool.tile([128, 512], F16, tag="w2", name="w2", bufs=2)
                nc.vector.stream_shuffle(w2, w_, shuf_mask)
                dst = dst_sb[j][:, tqb * 512:(tqb + 1) * 512]
                # out = u + sgn*w2: rows R' = R*c - I*s, I' = I*c + R*s
                nc.vector.scalar_tensor_tensor(dst, w2, sgn_sb, u, mult, add)

            def v_group(tt):
                ps = psum.tile([128, 512], F32, tag="mm", name="mm_ps")
                for kc in range(NKC):
                    nc.tensor.matmul(
                        ps,
                        lhsT=xT_sb[kc][:, tt * 128:(tt + 1) * 128],
                        rhs=wv_sb[kc],
                        start=(kc == 0), stop=(kc == NKC - 1),
                    )
                dst = v_sb[tt].rearrange("p (h e) -> p h e", e=65)[:, :, 0:64]
                src = ps.rearrange("p (h e) -> p h e", e=64)
                nc.scalar.copy(dst, src)

            def attn_unit(jq, jp, dn_map, o_sbs, pop_filler):
                # heads (2jp, 2jp+1) for query window jq: QK -> exp -> mask ->
                # PV; denominators land striped in dn_sb at partition
                # 32*(l%4), column half l//4.
                n_tk = 4 * (jq + 1)
                o_ps = [psum.tile([65, 512], F32, tag="o", name="o_ps")
                        for _ in range(2)]
                for tg in range(n_tk // 2):
                    def _off(kb):
                        dband = kb - 4 * jq
                        return 128 * dband if dband > 0 else 0

                    s_ps = [psum.tile([128, 1024], F32, tag="s", name="s_ps")
                            for _ in range(2)]
                    for c in range(2):
                        kb = tg * 2 + c
                        off = _off(kb)
                        for hh in range(2):
                            sl = s_ps[hh][:, c * 512 + off:(c + 1) * 512]
                            hb = hh * 64
                            nc.tensor.matmul(
                                sl,
                                lhsT=kT_sb[jp][hb:hb + 64,
                                               kb * 128:(kb + 1) * 128],
                                rhs=qT_sb[jp][hb:hb + 64,
                                              jq * 512 + off:(jq + 1) * 512],
                                start=True, stop=True,
                            )
                    exp_t = []
                    offs = [_off(tg * 2), _off(tg * 2 + 1)]
                    for hh in range(2):
                        e = wk_pool.tile([128, 1024], F16, tag="expS",
                                         name="expS", bufs=3)
                        if offs[0] == 0 and offs[1] == 0:
                            nc.scalar.activation(e, s_ps[hh], Exp,
                                                 bias=bias_sb[:, 0:1],
                                                 scale=SCALE)
                        else:
                            for c in range(2):
                                sl = slice(c * 512 + offs[c], (c + 1) * 512)
                                nc.scalar.activation(e[:, sl], s_ps[hh][:, sl],
                                                     Exp,
                                                     bias=bias_sb[:, 0:1],
                                                     scale=SCALE)
                        exp_t.append(e)
                    for c in range(2):
                        kb = tg * 2 + c
                        dband = kb - 4 * jq
                        if 0 <= dband <= 3:
                            off = 128 * dband
                            for hh in range(2):
                                sl = exp_t[hh][:, c * 512 + off:
                                               c * 512 + off + 128]
                                nc.vector.tensor_mul(sl, sl, md0_sb)
                    for c in range(2):
                        kb = tg * 2 + c
                        off = _off(kb)
                        for hh in range(2):
                            l = jp * 2 + hh
                            nc.tensor.matmul(
                                o_ps[hh][:, off:512],
                                lhsT=v_sb[kb][:, l * 65:(l + 1) * 65],
                                rhs=exp_t[hh][:, c * 512 + off:(c + 1) * 512],
                                start=(kb == 0), stop=(kb == n_tk - 1),
                            )
                    pop_filler()
                for hh in range(2):
                    l = jp * 2 + hh
                    o_sb = wk_pool.tile([65, 512], F32, tag="osb",
                                        name="o_sb", bufs=10)
                    nc.vector.tensor_copy(o_sb, o_ps[hh])
                    o_sbs[l] = o_sb
                    dnt, dr = dn_map(l)
                    nc.sync.dma_start(out=dnt[dr:dr + 1, :],
                                      in_=o_sb[64:65, :])

            def norm_closures(jq, dn_sb, o_sbs, rds, heads, row0=0):
                # reciprocal over dn rows `heads`, scatter, then per head a
                # K=1 partition-broadcast matmul + normalizing DVE multiply
                out = []

                def recip_head():
                    nh = len(heads)
                    rd = wk_pool.tile([8, 512], F32, tag="rd", name="rd",
                                      bufs=2)
                    nc.vector.reciprocal(rd[0:nh, :], dn_sb[row0:row0 + nh, :])
                    rd16 = wk_pool.tile([8, 512], F16, tag="rd16",
                                        name="rd16", bufs=2)
                    nc.vector.tensor_copy(rd16[0:nh, :], rd[0:nh, :])
                    for r, l in enumerate(heads):
                        nc.sync.dma_start(
                            out=rds[32 * (l % 2):32 * (l % 2) + 1,
                                    (l // 2) * 512:(l // 2) * 512 + 512],
                            in_=rd16[r:r + 1, :])
                out.append(recip_head)

                def bc_mult(l):
                    prow = 32 * (l % 2)
                    pcol = (l // 2) * 512
                    bc_t = psum.tile([128, 512], F32, tag="mm", name="bc_ps")
                    rdb = bc_t[0:64, :]
                    nc.tensor.matmul(
                        rdb,
                        lhsT=ones_sb[prow:prow + 1, :],
                        rhs=rds[prow:prow + 1, pcol:pcol + 512],
                        start=True, stop=True,
                    )
                    at = attnT_sb[l // 2]
                    rbase = (l % 2) * 64
                    nc.vector.tensor_mul(
                        at[rbase:rbase + 64, jq * 512:(jq + 1) * 512],
                        o_sbs[l][0:64, :], rdb,
                    )
                for l in heads:
                    out.append(lambda l=l: bc_mult(l))
                return out

            def proj_closures(jq):
                out = []

                def proj_co(tt, co):
                    ps = psum.tile([128, 512], F32, tag="mm", name="mm_ps")
                    for fc in range(4):
                        nc.tensor.matmul(
                            ps,
                            lhsT=attnT_sb[fc][:, tt * 128:(tt + 1) * 128],
                            rhs=wp_sb[fc][:, co * 512:(co + 1) * 512],
                            start=(fc == 0), stop=(fc == 3),
                        )
                    ysb = wk_pool.tile([128, 512], F16, tag="ysb",
                                       name="ysb", bufs=2)
                    nc.vector.tensor_copy(ysb, ps)
                    nc.sync.dma_start(
                        out=y.ap()[tt * 128:(tt + 1) * 128,
                                   co * 512:(co + 1) * 512],
                        in_=ysb,
                    )
                for tt in range(4 * jq, 4 * jq + 4):
                    for co in range(2):
                        out.append(lambda tt=tt, co=co: proj_co(tt, co))
                return out

            # ---- interleaved schedule: qkv tq-block c feeds attention
            # wave jq=c; the next block and the previous window's
            # normalize+proj drain as per-tg fillers inside each wave so the
            # PE never idles (keeps the clock ramp hot). Block 3's j1-j3
            # q/k groups defer into wave 3 (prioritized ahead of wave-2
            # normalize) to keep the long last wave fed; wave 3's own
            # normalize splits so heads 0-3 hide inside units (3,2..3). ----
            def qk_closures(tqb, js):
                out = []
                for j in js:
                    out.append(lambda j=j: qk_group(wq_sb, qT_sb, j, tqb))
                    out.append(lambda j=j: qk_group(wk_sb, kT_sb, j, tqb))
                return out

            def v_closures(tqb):
                return [lambda tt=tt: v_group(tt)
                        for tt in range(4 * tqb, 4 * tqb + 4)]

            for f in qk_closures(0, range(4)) + v_closures(0):
                f()

            pending = {}
            rds_t = {}
            for jq in range(NTQ):
                if jq == 0:
                    fillers = qk_closures(1, range(4)) + v_closures(1)
                elif jq == 1:
                    fillers = (pending[0] + qk_closures(2, range(4))
                               + v_closures(2))
                elif jq == 2:
                    fillers = (pending[1] + v_closures(3)
                               + qk_closures(3, [0]))
                else:
                    fillers = qk_closures(3, [1, 2, 3]) + pending[2]
                n_tgs = 8 * (jq + 1)
                state = {"idx": 0, "acc": 0.0, "n_tgs": n_tgs}

                def pop_filler(state=state, fillers=fillers):
                    # Bresenham spread: emit fillers evenly across the wave
                    state["acc"] += len(fillers) / state["n_tgs"]
                    while state["acc"] >= 1.0 and state["idx"] < len(fillers):
                        fillers[state["idx"]]()
                        state["idx"] += 1
                        state["acc"] -= 1.0

                dn_sb = wk_pool.tile([8, 512], F32, tag="dn", name="dn_sb",
                                     bufs=2)
                if jq == NTQ - 1:
                    dn_sb2 = wk_pool.tile([2, 512], F32, tag="dn2",
                                          name="dn_sb2", bufs=1)
                    dn_sb3 = wk_pool.tile([2, 512], F32, tag="dn3",
                                          name="dn_sb3", bufs=1)
                    dn_map = lambda l: ((dn_sb, l) if l < 4
                                        else (dn_sb2, l - 4) if l < 6
                                        else (dn_sb3, l - 6))
                else:
                    dn_map = lambda l: (dn_sb, l)
                o_sbs = {}
                rds = wk_pool.tile([33, 2048], F16, tag="rds", name="rds",
                                   bufs=2)
                for jp in range(4):
                    attn_unit(jq, jp, dn_map, o_sbs, pop_filler)
                    if jq == NTQ - 1 and jp == 1:
                        # mid-wave normalize of heads 0-3 rides the
                        # remaining units' tg stream
                        fillers.extend(
                            norm_closures(jq, dn_sb, o_sbs, rds, [0, 1, 2, 3]))
                    if jq == NTQ - 1 and jp == 2:
                        fillers.extend(
                            norm_closures(jq, dn_sb2, o_sbs, rds, [4, 5]))
                while state["idx"] < len(fillers):
                    fillers[state["idx"]]()
                    state["idx"] += 1
                if jq < NTQ - 1:
                    pending[jq] = (norm_closures(jq, dn_sb, o_sbs, rds,
                                                 list(range(8)))
                                   + proj_closures(jq))
                else:
                    for f in norm_closures(jq, dn_sb3, o_sbs, rds,
                                           [6, 7]):
                        f()
                    for f in proj_closures(jq):
                        f()
    return nc


_NC = None


def _get_nc():
    global _NC
    if _NC is None:
        _NC = _build_program()
    return _NC


def _rope_perm():
    """Row permutation applied to Wq/Wk rows (local feature order).

    Per head (64 contiguous rows, so QK^T is a single K=64 matmul):
    [R(freq 0..15), I(freq 0..15), R(freq 16..31), I(freq 16..31)] --
    R/I pairs sit 16 rows apart inside each 32-row block, which is what
    the DVE stream_shuffle (intra-32-block permute) needs for RoPE.
    """
    perm = []
    for hh in range(8):
        base = hh * HEAD_DIM
        perm.extend(base + 2 * i for i in range(16))
        perm.extend(base + 2 * i + 1 for i in range(16))
        perm.extend(base + 2 * i for i in range(16, 32))
        perm.extend(base + 2 * i + 1 for i in range(16, 32))
    return np.asarray(perm)


def _freq_rows():
    """freq index feeding each of the 128 cosP/sinP rows (2 head blocks)."""
    blk = np.array([*range(16), *range(16), *range(16, 32), *range(16, 32)])
    return np.concatenate([blk, blk])


def _core_in_map(c, x, w_attn, w_proj, cosP, sinP, perm):
    b, g = c // 2, c % 2
    heads = slice(g * F, (g + 1) * F)          # global q/k/v row block
    wq_rows = w_attn[0:C][heads][perm]          # (512, 1024) permuted
    wk_rows = w_attn[C:2 * C][heads][perm]
    wv_rows = w_attn[2 * C:3 * C][heads]        # natural order
    p = np.arange(128)[:, None]
    md0 = (np.arange(128)[None, :] >= p).astype(np.float16)
    return {
        "xT": np.ascontiguousarray(x[b].T).astype(np.float16),
        "wq": np.ascontiguousarray(wq_rows.T).astype(np.float16),
        "wk": np.ascontiguousarray(wk_rows.T).astype(np.float16),
        "wv": np.ascontiguousarray(wv_rows.T).astype(np.float16),
        "wp": np.ascontiguousarray(w_proj[:, heads].T).astype(np.float16),
        "cosP": cosP,
        "sinP": sinP,
        "md0": md0,
        "sgn": np.tile(np.repeat(np.float32([-1.0, 1.0]), 16), 4)[:, None],
    }


def _host_prep(freqs_cos, freqs_sin):
    """Pre-expanded 128-row fp16 rope tables (same values the baseline
    expanded on-device)."""
    rows = _freq_rows()
    cosP = np.ascontiguousarray(np.asarray(freqs_cos).T[rows]).astype(np.float16)
    sinP = np.ascontiguousarray(np.asarray(freqs_sin).T[rows]).astype(np.float16)
    return cosP, sinP, _rope_perm()


def kernel(x, freqs_cos, freqs_sin, w_attn, w_proj):
    nc = _get_nc()

    cosP, sinP, perm = _host_prep(freqs_cos, freqs_sin)
    in_maps = [
        _core_in_map(c, x, w_attn, w_proj, cosP, sinP, perm)
        for c in range(N_CORES)
    ]

    global _last_in_maps
    _last_in_maps = in_maps
    res = bass_utils.run_bass_kernel_spmd(nc, in_maps, core_ids=list(range(N_CORES)))

    out = np.empty((B, T, C), dtype=np.float32)
    for b in range(B):
        out[b] = (res.results[2 * b]["y"].astype(np.float32)
                  + res.results[2 * b + 1]["y"].astype(np.float32))
    return out
